# revision 14
# baseline (speedup 1.0000x reference)
"""Trainium2 Bass kernel for nn_AddMaskHead (ROI mask head: bilinear pool + concat + conv3x3 + BN + ReLU).

Self-contained: hardcodes shapes B=2, N=256 (512 boxes), C=256, H=96, W=128, P=14.
Shards data-parallel over the 512 boxes across 8 NeuronCores (64 boxes/core; each
core's boxes all come from a single image, so each core only needs its image's
features).
"""

import sys, os, types

sys.path.insert(0, "/opt/trn_rl_repo")

import numpy as np
import concourse.bass as bass
import concourse.mybir as mybir
import concourse.tile as tile
from concourse import bacc
from concourse.masks import make_identity

F32 = mybir.dt.float32
BF16 = mybir.dt.bfloat16

N_CORES = 8
NB = 64            # boxes per core
BATCH = 8          # boxes per inner batch
NBATCH = NB // BATCH
P = 14             # pooler resolution
C = 256            # channels
H, W = 96, 128     # feature map
PQ = P * P         # 196
Q0 = 128           # q-chunk 0 size (q = flattened (y,x) source pixel index)
Q1 = PQ - Q0       # 68


def _axis_static(in_s, out_s=P):
    # mirrors reference._resize_bilinear axis() in exact f32 arithmetic
    s = (np.arange(out_s, dtype=np.float32) + np.float32(0.5)) * np.float32(in_s / out_s) - np.float32(0.5)
    s = np.maximum(s, np.float32(0.0))
    i0 = np.minimum(np.floor(s).astype(np.int32), in_s - 1)
    i1 = np.minimum(i0 + 1, in_s - 1)
    w = (s - i0.astype(np.float32)).astype(np.float32)
    return i0, i1, w


YS0, YS1, WYS = _axis_static(H)
XS0, XS1, WXS = _axis_static(W)


def _consts_p():
    # per-partition constants: [128, 4] = (yv_q0, xv_q0, yv_q1, xv_q1); -1 pads
    arr = np.full((128, 4), -1.0, dtype=np.float32)
    for p in range(128):
        arr[p, 0] = (p // P)
        arr[p, 1] = (p % P)
    for p in range(Q1):
        q = Q0 + p
        arr[p, 2] = (q // P)
        arr[p, 3] = (q % P)
    return arr


def _consts_f():
    # free-dim constants (broadcast to all partitions on device):
    # [0:14]  wys, [14:28] 1-wys, [28:42] jc = arange(14)+0.5
    arr = np.zeros((1, 48), dtype=np.float32)
    arr[0, 0:14] = WYS
    arr[0, 14:28] = np.float32(1.0) - WYS
    arr[0, 28:42] = np.arange(P, dtype=np.float32) + np.float32(0.5)
    return arr


def build_kernel():
    nc = bacc.Bacc(None)

    feat = nc.declare_dram_parameter("features", [C, H, W], F32, isOutput=False)
    boxes = nc.declare_dram_parameter("boxes", [NB, 4], F32, isOutput=False)
    mask = nc.declare_dram_parameter("mask", [NB, C, P, P], F32, isOutput=False)
    wt_d = nc.declare_dram_parameter("wt", [128, 4, 9, 256], F32, isOutput=False)
    epi_d = nc.declare_dram_parameter("epi", [128, 5, 2], F32, isOutput=False)
    cp_d = nc.declare_dram_parameter("consts_p", [128, 4], F32, isOutput=False)
    cf_d = nc.declare_dram_parameter("consts_f", [1, 48], F32, isOutput=False)
    out_d = nc.declare_dram_parameter("out", [NB, C, P, P], F32, isOutput=True)

    mask_v = mask.rearrange("n (ch cp) i j -> cp ch n (i j)", cp=128)
    out_v = out_d.rearrange("n (oh op) i j -> op oh n (i j)", op=128)
    feat_v = feat.rearrange("(ch cp) h w -> cp ch h w", cp=128)

    with tile.TileContext(nc) as tc:
        with tc.tile_pool(name="persist", bufs=1) as pp:
            # ---------- persistent tiles ----------
            Wt = pp.tile([128, 4, 9, 256], BF16, tag="Wt")
            cfT = [pp.tile([128, 256], BF16, tag=f"cfT{qc}", name=f"cfT{qc}") for qc in range(2)]
            Xb = [pp.tile([128, 4, BATCH, 16, 16], BF16, tag=f"xbuf{i}", name=f"xbuf{i}")
                  for i in range(2)]
            ident = pp.tile([128, 128], F32, tag="ident")
            cpt = pp.tile([128, 4], F32, tag="cpt")          # yv/xv per-partition consts
            cft = pp.tile([128, 48], F32, tag="cft")         # free consts, replicated
            epi = pp.tile([128, 5, 2], F32, tag="epi")
            scale_e = pp.tile([128, 2], F32, tag="scale_e")
            bias_e = pp.tile([128, 2], F32, tag="bias_e")
            # per-box interpolation data (resident): [128, 64, 14] each
            Y0 = pp.tile([128, NB, P], F32, tag="Y0")
            Y1 = pp.tile([128, NB, P], F32, tag="Y1")
            WY = pp.tile([128, NB, P], F32, tag="WY")
            OWY = pp.tile([128, NB, P], F32, tag="OWY")
            X0 = pp.tile([128, NB, P], F32, tag="X0")
            X1 = pp.tile([128, NB, P], F32, tag="X1")
            WX = pp.tile([128, NB, P], F32, tag="WX")
            OWX = pp.tile([128, NB, P], F32, tag="OWX")

            make_identity(nc, ident[:])
            nc.sync.dma_start(cpt[:], cp_d[:])
            nc.vector.memset(Xb[0][:], 0.0)
            nc.vector.memset(Xb[1][:], 0.0)

            # ---------- phase 0 ----------
            with tc.tile_pool(name="ph0", bufs=1) as p0, \
                 tc.tile_pool(name="ph0db", bufs=2) as p0db, \
                 tc.tile_pool(name="ps0", bufs=2, space="PSUM") as ps0:

                # broadcast free-dim consts to all partitions
                cf1 = p0.tile([1, 48], F32, tag="cf1")
                nc.sync.dma_start(cf1[:], cf_d[:])
                nc.gpsimd.partition_broadcast(cft[:], cf1[:])

                # --- weights: DMA f32 (host-laid-out) + cast to bf16, ci-chunk at a time
                for ci in range(4):
                    wst = p0db.tile([128, 9 * 256], F32, tag="wst")
                    nc.sync.dma_start(wst[:], wt_d[:, ci].rearrange("p a b -> p (a b)"))
                    nc.vector.tensor_copy(Wt[:, ci].rearrange("p a b -> p (a b)"), wst[:])

                # --- epilogue scalars
                nc.sync.dma_start(epi[:].rearrange("p a b -> p (a b)"),
                                  epi_d.rearrange("p a b -> p (a b)"))
                tmp_e = p0.tile([128, 2], F32, tag="tmp_e")
                eps_t = p0.tile([128, 1], F32, tag="eps_t")
                nc.vector.memset(eps_t[:], 1e-5)
                nc.scalar.activation(tmp_e[:], epi[:, 4, :], mybir.ActivationFunctionType.Sqrt,
                                     bias=eps_t[:], scale=1.0)
                nc.vector.reciprocal(scale_e[:], tmp_e[:])
                nc.vector.tensor_mul(scale_e[:], scale_e[:], epi[:, 1, :])
                nc.vector.tensor_sub(bias_e[:], epi[:, 0, :], epi[:, 3, :])
                nc.vector.tensor_mul(bias_e[:], bias_e[:], scale_e[:])
                nc.vector.tensor_add(bias_e[:], bias_e[:], epi[:, 2, :])

                # --- concat-features (cf): static bilinear resize of this image's features
                R0 = p0.tile([128, 2, P, W], F32, tag="R0")
                R1 = p0.tile([128, 2, P, W], F32, tag="R1")
                for ch in range(2):
                    for i in range(P):
                        nc.sync.dma_start(R0[:, ch, i], feat_v[:, ch, int(YS0[i])])
                        nc.sync.dma_start(R1[:, ch, i], feat_v[:, ch, int(YS1[i])])
                rows = p0.tile([128, 2, P, W], F32, tag="rows")
                wys_b = cft[:, 0:14]
                owys_b = cft[:, 14:28]
                nc.vector.tensor_tensor(rows[:], R0[:], owys_b[:, None, :, None].to_broadcast([128, 2, P, W]),
                                        mybir.AluOpType.mult)
                nc.vector.tensor_tensor(R1[:], R1[:], wys_b[:, None, :, None].to_broadcast([128, 2, P, W]),
                                        mybir.AluOpType.mult)
                nc.vector.tensor_add(rows[:], rows[:], R1[:])

                cfv = p0.tile([128, 2, P, P], F32, tag="cfv")
                tmpj = p0.tile([128, 2, P], F32, tag="tmpj")
                for j in range(P):
                    nc.vector.tensor_scalar_mul(cfv[:, :, :, j], rows[:, :, :, int(XS0[j])],
                                                float(np.float32(1.0) - WXS[j]))
                    nc.vector.tensor_scalar_mul(tmpj[:], rows[:, :, :, int(XS1[j])], float(WXS[j]))
                    nc.vector.tensor_add(cfv[:, :, :, j], cfv[:, :, :, j], tmpj[:])

                # cfT[q, c] via PE transpose (f32 -> bf16 on copy-out)
                cfv_f = cfv[:].rearrange("p c i j -> p c (i j)")
                nc.vector.memset(cfT[1][:], 0.0)
                for ch in range(2):
                    pst = ps0.tile([128, 128], F32, tag="pst")
                    nc.tensor.transpose(pst[:], cfv_f[:, ch, 0:128], ident[:])
                    nc.scalar.copy(cfT[0][:, ch * 128 : ch * 128 + 128], pst[:])
                    pst2 = ps0.tile([Q1, 128], F32, tag="pst2")
                    nc.tensor.transpose(pst2[:], cfv_f[:, ch, 128:196], ident[:])
                    nc.scalar.copy(cfT[1][0:Q1, ch * 128 : ch * 128 + 128], pst2[:])

                # --- box math (all [128, ...] replicated across partitions)
                bx1 = p0.tile([1, NB * 4], F32, tag="bx1")
                nc.sync.dma_start(bx1[:], boxes.rearrange("n f -> (n f)")[None, :])
                bxb = p0.tile([128, NB, 4], F32, tag="bxb")
                nc.gpsimd.partition_broadcast(bxb[:].rearrange("p n f -> p (n f)"), bx1[:])

                # floor(x) for x>=0: r = round_to_nearest(x) via int32 cast; r -= (r > x)
                t = p0.tile([128, NB, 4], F32, tag="t")
                fr = p0.tile([128, NB, 4], F32, tag="fr")
                ti32 = p0.tile([128, NB, 4], mybir.dt.int32, tag="ti32")
                nc.vector.tensor_scalar_mul(t[:], bxb[:], 0.125)
                nc.vector.tensor_copy(ti32[:], t[:])
                nc.vector.tensor_copy(fr[:], ti32[:])
                nc.vector.tensor_tensor(ti32[:].bitcast(F32), fr[:], t[:], mybir.AluOpType.is_gt)
                nc.vector.tensor_sub(t[:], fr[:], ti32[:].bitcast(F32))  # t = floor(bx*0.125)

                # border adjust per axis: a -= nlt*(b==14); b += nlt*(b!=14)
                ab = p0.tile([128, NB, 4], F32, tag="ab")   # xa, ya, xb, yb (adjusted)
                d = p0.tile([128, NB], F32, tag="d")
                nlt = p0.tile([128, NB], F32, tag="nlt")
                beq = p0.tile([128, NB], F32, tag="beq")
                adj = p0.tile([128, NB], F32, tag="adj")
                for ax in range(2):  # 0: x (cols 0,2), 1: y (cols 1,3)
                    a_in, b_in = t[:, :, ax], t[:, :, 2 + ax]
                    a_o, b_o = ab[:, :, ax], ab[:, :, 2 + ax]
                    nc.vector.tensor_sub(d[:], b_in, a_in)
                    nc.vector.tensor_scalar(nlt[:], d[:], 1.0, None, mybir.AluOpType.is_lt)
                    nc.vector.tensor_scalar(beq[:], b_in, float(P), None, mybir.AluOpType.is_equal)
                    nc.vector.tensor_mul(adj[:], nlt[:], beq[:])
                    nc.vector.tensor_sub(a_o, a_in, adj[:])
                    nc.vector.tensor_add(b_o, b_in, nlt[:])
                    nc.vector.tensor_sub(b_o, b_o, adj[:])

                # per-axis interpolation indices/weights: [128, NB, 14]
                jc_b = cft[:, 28:42]
                nwid = p0.tile([128, NB], F32, tag="nwid")
                him1 = p0.tile([128, NB], F32, tag="him1")
                s = p0.tile([128, NB, P], F32, tag="s")
                frs = p0.tile([128, NB, P], F32, tag="frs")
                si32 = p0.tile([128, NB, P], mybir.dt.int32, tag="si32")
                i0c = p0.tile([128, NB, P], F32, tag="i0c")
                i1c = p0.tile([128, NB, P], F32, tag="i1c")
                RC14 = float(np.float32(1.0) / np.float32(P))
                for ax, (lo_c, I0, I1, Wf, OWf) in enumerate(
                    [(0, X0, X1, WX, OWX), (1, Y0, Y1, WY, OWY)]
                ):
                    nc.vector.tensor_sub(nwid[:], ab[:, :, 2 + ax], ab[:, :, ax])
                    nc.vector.tensor_scalar_sub(him1[:], nwid[:], 1.0)
                    # s = max(jc * n / 14 - 0.5, 0)
                    nc.vector.tensor_tensor(s[:], nwid[:, :, None].to_broadcast([128, NB, P]),
                                            jc_b[:, None, :].to_broadcast([128, NB, P]),
                                            mybir.AluOpType.mult)
                    nc.vector.tensor_scalar(s[:], s[:], RC14, -0.5,
                                            mybir.AluOpType.mult, mybir.AluOpType.add)
                    nc.vector.tensor_scalar(s[:], s[:], 0.0, None, mybir.AluOpType.max)
                    # i0 = min(floor(s), n-1); i1 = min(i0+1, n-1); w = s - i0
                    nc.vector.tensor_copy(si32[:], s[:])
                    nc.vector.tensor_copy(frs[:], si32[:])
                    nc.vector.tensor_tensor(si32[:].bitcast(F32), frs[:], s[:], mybir.AluOpType.is_gt)
                    nc.vector.tensor_sub(i0c[:], frs[:], si32[:].bitcast(F32))
                    nc.vector.tensor_tensor(i0c[:], i0c[:], him1[:, :, None].to_broadcast([128, NB, P]),
                                            mybir.AluOpType.min)
                    nc.vector.tensor_scalar_add(i1c[:], i0c[:], 1.0)
                    nc.vector.tensor_tensor(i1c[:], i1c[:], him1[:, :, None].to_broadcast([128, NB, P]),
                                            mybir.AluOpType.min)
                    nc.vector.tensor_sub(Wf[:], s[:], i0c[:])
                    nc.vector.tensor_scalar(OWf[:], Wf[:], -1.0, 1.0,
                                            mybir.AluOpType.mult, mybir.AluOpType.add)
                    # absolute source index = lo + i
                    nc.vector.tensor_tensor(I0[:], i0c[:], ab[:, :, lo_c][:, :, None].to_broadcast([128, NB, P]),
                                            mybir.AluOpType.add)
                    nc.vector.tensor_tensor(I1[:], i1c[:], ab[:, :, lo_c][:, :, None].to_broadcast([128, NB, P]),
                                            mybir.AluOpType.add)

            # ---------- main loop ----------
            with tc.tile_pool(name="loop", bufs=2) as lp, \
                 tc.tile_pool(name="gpool", bufs=2) as gp, \
                 tc.tile_pool(name="psc", bufs=4, space="PSUM") as psc, \
                 tc.tile_pool(name="psv", bufs=4, space="PSUM") as psv:

                for b in range(NBATCH):
                    n0 = b * BATCH
                    X = Xb[b % 2]

                    # --- mask features: DMA f32 + cast into padded X interior
                    mst = lp.tile([128, 2, BATCH, PQ], F32, tag="mst")
                    for ch in range(2):
                        nc.sync.dma_start(mst[:, ch], mask_v[:, ch, n0 : n0 + BATCH])
                    mst_v = mst[:].rearrange("p ch n (i j) -> p ch n i j", j=P)
                    for ch in range(2):
                        nc.vector.tensor_copy(X[:, ch, :, 1:15, 1:15], mst_v[:, ch])

                    # --- G matrices for this batch: [128(q), BATCH, 14, 14] bf16, per q-chunk
                    Gt = []
                    for qc in range(2):
                        yv = cpt[:, 2 * qc : 2 * qc + 1, None].to_broadcast([128, BATCH, P])
                        xv = cpt[:, 2 * qc + 1 : 2 * qc + 2, None].to_broadcast([128, BATCH, P])
                        my = gp.tile([128, BATCH, P], F32, tag=f"my{qc}")
                        mx = gp.tile([128, BATCH, P], F32, tag=f"mx{qc}")
                        cmp = gp.tile([128, BATCH, P], F32, tag=f"cmp{qc}")
                        bsl = (slice(None), slice(n0, n0 + BATCH), slice(None))
                        nc.vector.tensor_tensor(my[:], Y0[bsl], yv, mybir.AluOpType.is_equal)
                        nc.vector.tensor_mul(my[:], my[:], OWY[bsl])
                        nc.vector.tensor_tensor(cmp[:], Y1[bsl], yv, mybir.AluOpType.is_equal)
                        nc.vector.tensor_mul(cmp[:], cmp[:], WY[bsl])
                        nc.vector.tensor_add(my[:], my[:], cmp[:])
                        nc.vector.tensor_tensor(mx[:], X0[bsl], xv, mybir.AluOpType.is_equal)
                        nc.vector.tensor_mul(mx[:], mx[:], OWX[bsl])
                        nc.vector.tensor_tensor(cmp[:], X1[bsl], xv, mybir.AluOpType.is_equal)
                        nc.vector.tensor_mul(cmp[:], cmp[:], WX[bsl])
                        nc.vector.tensor_add(mx[:], mx[:], cmp[:])
                        G = gp.tile([128, BATCH, P, P], BF16, tag=f"G{qc}")
                        nc.vector.tensor_tensor(G[:], my[:, :, :, None].to_broadcast([128, BATCH, P, P]),
                                                mx[:, :, None, :].to_broadcast([128, BATCH, P, P]),
                                                mybir.AluOpType.mult)
                        Gt.append(G)

                    # --- crops = cfT^T @ G  -> X interior (bf16)
                    for cc in range(2):
                        pcs = []
                        for pr in range(4):
                            ps = psc.tile([128, 2 * PQ], F32, tag="crops", name=f"crops_{b}_{cc}_{pr}")
                            pcs.append(ps)
                            for qc in range(2):
                                nc.tensor.matmul(
                                    ps[:],
                                    cfT[qc][:, cc * 128 : cc * 128 + 128],
                                    Gt[qc][:, 2 * pr : 2 * pr + 2].rearrange("p n i j -> p (n i j)"),
                                    start=(qc == 0), stop=(qc == 1),
                                )
                        for pr in range(4):
                            nc.scalar.copy(
                                X[:, 2 + cc, 2 * pr : 2 * pr + 2, 1:15, 1:15],
                                pcs[pr][:].rearrange("p (n i j) -> p n i j", n=2, i=P),
                            )

                    # --- conv 3x3 (+BN+ReLU) over cin=512 via 9 shifted matmuls
                    ost = lp.tile([128, 2, BATCH, PQ], F32, tag="ost")
                    for oc in range(2):
                        pcv = []
                        for pr in range(4):
                            pcv.append(psv.tile([128, 2 * PQ], F32, tag="conv",
                                                name=f"conv_{b}_{oc}_{pr}"))
                        for ci in range(4):
                            for sdy in range(3):
                                for sdx in range(3):
                                    first = (ci == 0 and sdy == 0 and sdx == 0)
                                    last = (ci == 3 and sdy == 2 and sdx == 2)
                                    lhsT = Wt[:, ci, 3 * sdy + sdx, oc * 128 : oc * 128 + 128]
                                    for pr in range(4):
                                        rhs = X[:, ci, 2 * pr : 2 * pr + 2, sdy : sdy + P, sdx : sdx + P]
                                        nc.tensor.matmul(
                                            pcv[pr][:],
                                            lhsT,
                                            rhs,
                                            start=first, stop=last,
                                        )
                        for pr in range(4):
                            nc.scalar.activation(
                                ost[:, oc, 2 * pr : 2 * pr + 2],
                                pcv[pr][:].rearrange("p (n q) -> p n q", n=2),
                                mybir.ActivationFunctionType.Relu,
                                bias=bias_e[:, oc : oc + 1],
                                scale=scale_e[:, oc : oc + 1],
                            )
                    for oh in range(2):
                        nc.sync.dma_start(out_v[:, oh, n0 : n0 + BATCH], ost[:, oh])

    nc.compile()
    return nc


# ---------------------------------------------------------------------------
# host-side sharding / unsharding
# ---------------------------------------------------------------------------

def _prep_in_maps(features, proposal_boxes, mask_features, conv_w, conv_b,
                  bn_gamma, bn_beta, bn_mean, bn_var):
    features = np.asarray(features, dtype=np.float32)
    proposal_boxes = np.asarray(proposal_boxes, dtype=np.float32)
    mask_features = np.asarray(mask_features, dtype=np.float32)
    conv_w = np.asarray(conv_w, dtype=np.float32)
    # weight layout: [cout=256, cin=512, 3, 3] -> [cin_par=128, cin_hi=4, 9, cout=256]
    wt = np.ascontiguousarray(
        conv_w.reshape(256, 4, 128, 3 * 3).transpose(2, 1, 3, 0)
    ).astype(np.float32)
    epi = np.stack([np.asarray(x, dtype=np.float32) for x in
                    (conv_b, bn_gamma, bn_beta, bn_mean, bn_var)])  # [5, 256]
    epi = np.ascontiguousarray(epi.reshape(5, 2, 128).transpose(2, 0, 1)).astype(np.float32)
    cp = _consts_p()
    cfc = _consts_f()

    in_maps = []
    for i in range(N_CORES):
        img = i // (N_CORES // 2)
        n0 = (i * NB) % 256
        in_maps.append({
            "features": np.ascontiguousarray(features[img]),
            "boxes": np.ascontiguousarray(proposal_boxes[img, n0 : n0 + NB]),
            "mask": np.ascontiguousarray(mask_features[i * NB : (i + 1) * NB]),
            "wt": wt,
            "epi": epi,
            "consts_p": cp,
            "consts_f": cfc,
        })
    return in_maps


_NC_CACHE = {}


def _get_nc():
    if "nc" not in _NC_CACHE:
        _NC_CACHE["nc"] = build_kernel()
    return _NC_CACHE["nc"]


def _install_ntff_shim():
    """antenv.axon_hooks is missing in this image; shim it so trace=True works."""
    try:
        import antenv
        if hasattr(antenv, "axon_hooks"):
            return
        from trn_agent_boot.trn_boot import _ntff_profile_via_ctypes
        mod = types.ModuleType("antenv.axon_hooks")
        _h = [None]
        mod.set_axon_ntff_profile_hook = lambda h: _h.__setitem__(0, h)
        mod.get_axon_ntff_profile_hook = lambda: _h[0]
        sys.modules["antenv.axon_hooks"] = mod
        antenv.axon_hooks = mod
        mod.set_axon_ntff_profile_hook(_ntff_profile_via_ctypes("/opt/axon/libaxon_pjrt.so"))
    except Exception:
        pass


def run(trace=False, tmpdir=None, **inputs):
    from concourse.bass_utils import run_bass_kernel_spmd

    if trace:
        _install_ntff_shim()
    nc = _get_nc()
    in_maps = _prep_in_maps(**inputs)
    res = run_bass_kernel_spmd(nc, in_maps, core_ids=list(range(N_CORES)),
                               trace=trace, tmpdir=tmpdir)
    out = np.concatenate([np.asarray(res.results[i]["out"]) for i in range(N_CORES)], axis=0)
    return out.astype(np.float32), res


def kernel(**inputs):
    out, _ = run(trace=False, **inputs)
    return out
